# revision 3
# baseline (speedup 1.0000x reference)
# Multi-head causal attention (B=4, T=2048, D=1024, H=16) on 8 TRN2 NeuronCores.
#
# Sharding (tensor-parallel over heads, data-parallel over batch):
# core = (batch b = core//2, head-half hh = core%2). Each core projects
# Q/K/V for its 8 heads over the FULL sequence (no duplicated projection
# work), runs causal attention for those heads, then the two cores of a
# batch exchange bf16 attention outputs with a pairwise AllGather (256KB
# per 256-token chunk, 8 chunks, overlapped with compute) and each core
# computes the output projection for its 512 output columns over all
# 2048 tokens. Fully SPMD-symmetric: all rank differences live in the
# input data (weight slices), never in control flow.
#
# Math simplifications (exact under softmax):
#   - K bias dropped: (q+bq)&(k+bk) differs from (q+bq)&k by a value
#     constant along the softmax axis -> cancels.
#   - V bias folded into the output bias on the host: bo' = bo + wo@bv.
#
# Per-core kernel (all matmul operands bf16, fp32 PSUM accumulation):
#   qT/kT = W @ x^T per head-pair group; V kept (t, d)-major with an extra
#   ones column so the attention-value matmul also produces the softmax
#   denominator. Scores computed transposed (tk partition, tq free), exp on
#   ScalarE (no max subtraction: |scores| <= ~3), causal masking only on
#   the diagonal block-pair via one multiplicative {0,1} mask tile,
#   denominator broadcast across partitions on GpSimd.
import numpy as np
import ml_dtypes

B, T, D, H, DH, P = 4, 2048, 1024, 16, 64, 128
DHALF = 512        # head-half width (8 heads)
NCORES = 8
BF16 = ml_dtypes.bfloat16

_COMPILED = {}


def _build_nc():
    from contextlib import ExitStack
    from functools import partial
    import concourse.mybir as mybir
    import concourse.tile as tile
    from concourse import bacc

    bf = mybir.dt.bfloat16
    f32 = mybir.dt.float32
    EXP = mybir.ActivationFunctionType.Exp

    nc = bacc.Bacc("TRN2", target_bir_lowering=False, debug=False,
                   num_devices=NCORES)

    # ---- DRAM I/O ----
    x_d = nc.dram_tensor("xT", [D, T], bf, kind="ExternalInput").ap()
    wq_d = nc.dram_tensor("wqT", [D, DHALF], bf, kind="ExternalInput").ap()
    wk_d = nc.dram_tensor("wkT", [D, DHALF], bf, kind="ExternalInput").ap()
    wv_d = nc.dram_tensor("wvT", [D, DHALF], bf, kind="ExternalInput").ap()
    wo_d = nc.dram_tensor("woT", [D, DHALF], bf, kind="ExternalInput").ap()
    bq_d = nc.dram_tensor("bq_r", [P, 4], f32, kind="ExternalInput").ap()
    bo_d = nc.dram_tensor("bo_r", [P, 4], f32, kind="ExternalInput").ap()
    cm_d = nc.dram_tensor("cmask", [P, 1024], bf, kind="ExternalInput").ap()
    y_d = nc.dram_tensor("yT", [DHALF, T], f32, kind="ExternalOutput").ap()

    x_r = x_d.rearrange("(g p) t -> p g t", p=P)

    with tile.TileContext(nc) as tc, ExitStack() as ctx:
        const = ctx.enter_context(tc.tile_pool(name="const", bufs=1))
        xchunk = ctx.enter_context(tc.tile_pool(name="xchunk", bufs=2))
        expps = ctx.enter_context(tc.tile_pool(name="expps", bufs=4))
        small = ctx.enter_context(tc.tile_pool(name="small", bufs=2))
        afp = ctx.enter_context(tc.tile_pool(name="afp", bufs=4))
        dram = ctx.enter_context(tc.tile_pool(name="dram", bufs=3, space="DRAM"))
        ps_s = ctx.enter_context(tc.tile_pool(name="ps_s", bufs=2, space="PSUM"))
        ps_av = ctx.enter_context(tc.tile_pool(name="ps_av", bufs=1, space="PSUM"))
        ps_m = ctx.enter_context(tc.tile_pool(name="ps_m", bufs=2, space="PSUM"))

        # ---- resident SBUF tensors ----
        wq_sb = const.tile([P, 8, 4, P], bf)     # [k, kg, g, dout]
        wk_sb = const.tile([P, 8, 4, P], bf)
        wv_sb = const.tile([P, 8, DHALF], bf)    # [k, kg, d] (moving operand)
        wo_sb = const.tile([P, 8, 4, P], bf)     # [din, kg, o, dout]
        k_sb = const.tile([P, 4, T], bf)         # k^T per head-pair group
        q_sb = const.tile([P, 4, T], bf)
        v1_sb = const.tile([P, 16, 8, 65], bf)   # [tk, tkblk, head, V|1]
        a_sb = const.tile([P, 4, T], bf)         # attention out (d, tq)
        mk_sb = const.tile([P, 1024], bf)        # diagonal mask
        bq_sb = const.tile([P, 4], f32)
        bo_sb = const.tile([P, 4], f32)

        dma = nc.sync.dma_start
        nc.vector.memset(v1_sb[:, :, :, 64:65], 1.0)

        mm = nc.tensor.matmul

        def _v_group(xc, ts, ti):
            # one 128-token block of V, all 512 head-half dims
            t = 4 * ts + ti
            ps = ps_m.tile([P, DHALF], f32, name="ps", tag="ps")
            for kg in range(8):
                mm(ps, lhsT=xc[:, kg, ti * P:(ti + 1) * P],
                   rhs=wv_sb[:, kg, :], start=(kg == 0), stop=(kg == 7))
            nc.vector.tensor_copy(
                out=v1_sb[:, t, :, 0:64],
                in_=ps.rearrange("p (h c) -> p h c", c=64))

        def _k_group(xc, ts, g):
            ps = ps_m.tile([P, DHALF], f32, name="ps", tag="ps")
            for kg in range(8):
                mm(ps, lhsT=wk_sb[:, kg, g, :], rhs=xc[:, kg, :],
                   start=(kg == 0), stop=(kg == 7))
            nc.vector.tensor_copy(
                out=k_sb[:, g, ts * DHALF:(ts + 1) * DHALF], in_=ps)

        def _q_group(xc, ts, g):
            ps = ps_m.tile([P, DHALF], f32, name="ps", tag="ps")
            for kg in range(8):
                mm(ps, lhsT=wq_sb[:, kg, g, :], rhs=xc[:, kg, :],
                   start=(kg == 0), stop=(kg == 7))
            nc.vector.tensor_scalar_add(
                out=q_sb[:, g, ts * DHALF:(ts + 1) * DHALF], in0=ps,
                scalar1=bq_sb[:, g:g + 1])

        def proj_chunk_thunks(ts):
            # stream 512 tokens of x^T; V blocks 0-1 first (earliest need),
            # then K and Q groups, then V blocks 2-3.
            xc = xchunk.tile([P, 8, DHALF], bf, name="xc", tag="xc")
            dma(out=xc, in_=x_r[:, :, ts * DHALF:(ts + 1) * DHALF])
            th = [partial(_v_group, xc, ts, ti) for ti in (0, 1)]
            th += [partial(_k_group, xc, ts, g) for g in range(4)]
            th += [partial(_q_group, xc, ts, g) for g in range(4)]
            th += [partial(_v_group, xc, ts, ti) for ti in (2, 3)]
            return th

        def attn_slot(g, j):
            # one head-pair group, one 256-query slot, keys 0..2j+1 blocks.
            # One accumulator bank per head; row 64 collects the softmax
            # denominator via the ones column of v1_sb.
            pav = [ps_av.tile([65, 256], f32, tag=f"pav{c}",
                              name=f"pav{c}") for c in (0, 1)]
            last_k = 2 * j + 1
            for kk in range(j + 1):
                kb = (2 * kk, 2 * kk + 1)
                ps = ps_s.tile([P, 1024], f32, name="scps", tag="scps")
                expp = expps.tile([P, 1024], bf, name="expp", tag="expp")
                for c in (0, 1):         # head within pair
                    for i in (0, 1):     # tk block within pair
                        mm(ps[:, c * 512 + i * 256: c * 512 + i * 256 + 256],
                           lhsT=k_sb[64 * c:64 * c + 64, g,
                                     kb[i] * P:(kb[i] + 1) * P],
                           rhs=q_sb[64 * c:64 * c + 64, g,
                                    j * 256:(j + 1) * 256],
                           start=True, stop=True,
                           tile_position=(64 * c, 0))
                nc.scalar.activation(out=expp, in_=ps, func=EXP, scale=0.125)
                if kk == j:
                    nc.vector.tensor_mul(expp, expp, mk_sb)
                for c in (0, 1):
                    for i in (0, 1):
                        mm(pav[c],
                           lhsT=v1_sb[:, kb[i], 2 * g + c, :],
                           rhs=expp[:, c * 512 + i * 256:
                                    c * 512 + i * 256 + 256],
                           start=(kb[i] == 0), stop=(kb[i] == last_k))
            # copy accumulators to SBUF right away so the PSUM banks free up
            av = [small.tile([65, 256], f32, tag=f"av{c}", bufs=2,
                             name=f"av{c}") for c in (0, 1)]
            for c in (0, 1):
                nc.vector.tensor_copy(out=av[c], in_=pav[c])
            # both heads' denominators into one partition-base-0 tile
            # (reciprocal_approx_fast corrupts base!=0 inputs on HW)
            den2 = small.tile([1, 512], f32, tag="den2", bufs=1, name="den2")
            for c in (0, 1):
                nc.vector.tensor_copy(out=den2[:, c * 256:(c + 1) * 256],
                                      in_=av[c][64:65, :])
            rec = small.tile([1, 512], f32, tag="rec", bufs=1, name="rec")
            nc.vector.reciprocal_approx_fast(out=rec, in_=den2)
            sbb = small.tile([64, 512], f32, tag="sbb", name="sbb")
            nc.gpsimd.partition_broadcast(sbb, rec)
            for c in (0, 1):
                nc.vector.tensor_mul(
                    out=a_sb[64 * c:64 * c + 64, g, j * 256:(j + 1) * 256],
                    in0=av[c][0:64, :], in1=sbb[:, c * 256:(c + 1) * 256])

        af_tiles = {}

        def ag_chunk(j):
            # pairwise AllGather of this 256-token chunk of attention out:
            # both ranks end up with [2 ranks x 4 groups] = all 16 heads,
            # in global head order (matches unsliced woT row order).
            inb = dram.tile([P, 1024], bf, name="agin", tag="agin")
            outb = dram.tile([2, P, 1024], bf, name="agout", tag="agout")
            nc.gpsimd.dma_start(
                out=inb.rearrange("p (g q) -> p g q", q=256),
                in_=a_sb[:, :, j * 256:(j + 1) * 256])
            nc.gpsimd.collective_compute(
                "AllGather",
                mybir.AluOpType.bypass,
                replica_groups=[[0, 1], [2, 3], [4, 5], [6, 7]],
                ins=[inb.opt()],
                outs=[outb.opt()],
            )
            af = afp.tile([P, 8, 256], bf, name="af", tag="af")
            dma(out=af.rearrange("p (r g) q -> p r g q", r=2),
                in_=outb.rearrange("r p (g q) -> p r g q", q=256))
            af_tiles[j] = af

        def wo_group(ch, o):
            # 128 output rows x 256 tokens; contraction over all 16 heads
            af = af_tiles[ch]
            ps = ps_m.tile([P, DHALF], f32, name="ps", tag="ps")
            for kg in range(8):
                mm(ps[:, 0:256], lhsT=wo_sb[:, kg, o, :], rhs=af[:, kg, :],
                   start=(kg == 0), stop=(kg == 7))
            ysb = small.tile([P, 256], f32, tag="ysb", name="ysb")
            nc.vector.tensor_scalar_add(out=ysb, in0=ps[:, 0:256],
                                        scalar1=bo_sb[:, o:o + 1])
            dma(out=y_d[o * P:(o + 1) * P, ch * 256:(ch + 1) * 256], in_=ysb)

        def drain(pend, n):
            for _ in range(min(n, len(pend))):
                pend.pop(0)()

        # ---- emission order sets priority ----
        # Warm the exp table set while the first DMAs are in flight.
        warm = small.tile([1, 8], f32, tag="warm", bufs=1, name="warm")
        nc.vector.memset(warm, 0.0)
        nc.scalar.activation(out=warm, in_=warm, func=EXP)

        dma(out=wv_sb, in_=wv_d.rearrange("(kg p) d -> p kg d", p=P))
        pend0 = proj_chunk_thunks(0)           # fires the x chunk-0 DMA
        dma(out=wk_sb, in_=wk_d.rearrange("(kg p) (g c) -> p kg g c", p=P, c=P))
        dma(out=bq_sb, in_=bq_d)
        drain(pend0, 2)                        # V blocks 0-1 of chunk 0
        dma(out=wq_sb, in_=wq_d.rearrange("(kg p) (g c) -> p kg g c", p=P, c=P))
        drain(pend0, 4)                        # K of chunk 0
        dma(out=mk_sb, in_=cm_d)
        dma(out=bo_sb, in_=bo_d)
        drain(pend0, 4)                        # Q of chunk 0
        dma(out=wo_sb, in_=wo_d.rearrange("(kg p) (o c) -> p kg o c", p=P, c=P))

        pend = pend0 + proj_chunk_thunks(1) + proj_chunk_thunks(2) \
            + proj_chunk_thunks(3)
        wo_pend = []
        for j in range(8):
            for g in range(4):
                attn_slot(g, j)
                drain(pend, 2)
                drain(wo_pend, 1)
            ag_chunk(j)
            wo_pend += [partial(wo_group, j, o) for o in range(4)]
        drain(pend, 99)
        drain(wo_pend, 99)

    nc.compile()
    return nc


def _get_nc():
    if "nc" not in _COMPILED:
        _COMPILED["nc"] = _build_nc()
    return _COMPILED["nc"]


def _make_mask():
    tri = np.triu(np.ones((P, P), np.float32))   # keep tk <= tq
    on = np.ones((P, P), np.float32)
    z = np.zeros((P, P), np.float32)
    # expp column layout per head c: [tk 2j vs (q2j | q2j+1) | tk 2j+1 vs ...]
    row = np.concatenate([tri, on, z, tri] * 2, axis=1)
    return row.astype(BF16)


def _make_in_maps(x, wq, bq, wk, bk, wv, bv, wo, bo):
    bfc = lambda a: np.ascontiguousarray(np.asarray(a, np.float32)).astype(BF16)
    bo_eff = (np.asarray(bo, np.float32)
              + np.asarray(wo, np.float32) @ np.asarray(bv, np.float32))
    mask = _make_mask()
    in_maps = []
    for core in range(NCORES):
        b, hh = core // 2, core % 2
        sl = slice(DHALF * hh, DHALF * hh + DHALF)
        m = {
            "xT": bfc(np.asarray(x[b], np.float32).T),
            "wqT": bfc(np.asarray(wq, np.float32)[sl, :].T),
            "wkT": bfc(np.asarray(wk, np.float32)[sl, :].T),
            "wvT": bfc(np.asarray(wv, np.float32)[sl, :].T),
            "woT": bfc(np.asarray(wo, np.float32)[sl, :].T),
            "bq_r": np.ascontiguousarray(
                np.asarray(bq, np.float32)[sl].reshape(4, P).T),
            "bo_r": np.ascontiguousarray(bo_eff[sl].reshape(4, P).T),
            "cmask": mask,
        }
        in_maps.append(m)
    return in_maps


def _run(inputs, trace=False):
    from concourse.bass_utils import run_bass_kernel_spmd
    nc = _get_nc()
    in_maps = _make_in_maps(**inputs)
    res = run_bass_kernel_spmd(nc, in_maps, list(range(NCORES)), trace=trace)
    y = np.empty((B, T, D), np.float32)
    for core in range(NCORES):
        b, hh = core // 2, core % 2
        y[b][:, DHALF * hh:DHALF * hh + DHALF] = res.results[core]["yT"].T
    return y, res


def kernel(**inputs):
    y, _ = _run(inputs, trace=False)
    return y


# revision 5
# speedup vs baseline: 1.0227x; 1.0227x over previous
# Multi-head causal attention (B=4, T=2048, D=1024, H=16) on 8 TRN2 NeuronCores.
#
# Sharding (tensor-parallel over heads, data-parallel over batch):
# core = (batch b = core//2, head-half hh = core%2). Each core projects
# Q/K/V for its 8 heads over the FULL sequence (no duplicated projection
# work), runs causal attention for those heads, then the two cores of a
# batch exchange bf16 attention outputs with a pairwise AllGather (256KB
# per 256-token chunk, 8 chunks, overlapped with compute) and each core
# computes the output projection for its 512 output columns over all
# 2048 tokens. Fully SPMD-symmetric: all rank differences live in the
# input data (weight slices), never in control flow.
#
# Math simplifications (exact under softmax):
#   - K bias dropped: (q+bq)&(k+bk) differs from (q+bq)&k by a value
#     constant along the softmax axis -> cancels.
#   - V bias folded into the output bias on the host: bo' = bo + wo@bv.
#
# Per-core kernel (all matmul operands bf16, fp32 PSUM accumulation):
#   qT/kT = W @ x^T per head-pair group; V kept (t, d)-major with an extra
#   ones column so the attention-value matmul also produces the softmax
#   denominator. Scores computed transposed (tk partition, tq free), exp on
#   ScalarE (no max subtraction: |scores| <= ~3), causal masking only on
#   the diagonal block-pair via one multiplicative {0,1} mask tile,
#   denominator broadcast across partitions on GpSimd.
import numpy as np
import ml_dtypes

B, T, D, H, DH, P = 4, 2048, 1024, 16, 64, 128
DHALF = 512        # head-half width (8 heads)
NCORES = 8
BF16 = ml_dtypes.bfloat16

_COMPILED = {}


def _build_nc():
    from contextlib import ExitStack
    from functools import partial
    import concourse.mybir as mybir
    import concourse.tile as tile
    from concourse import bacc

    bf = mybir.dt.bfloat16
    f32 = mybir.dt.float32
    EXP = mybir.ActivationFunctionType.Exp

    nc = bacc.Bacc("TRN2", target_bir_lowering=False, debug=False,
                   num_devices=NCORES)

    # ---- DRAM I/O ----
    x_d = nc.dram_tensor("xT", [D, T], bf, kind="ExternalInput").ap()
    wq_d = nc.dram_tensor("wqT", [D, DHALF], bf, kind="ExternalInput").ap()
    wk_d = nc.dram_tensor("wkT", [D, DHALF], bf, kind="ExternalInput").ap()
    wv_d = nc.dram_tensor("wvT", [D, DHALF], bf, kind="ExternalInput").ap()
    wo_d = nc.dram_tensor("woT", [D, DHALF], bf, kind="ExternalInput").ap()
    bq_d = nc.dram_tensor("bq_r", [P, 4], f32, kind="ExternalInput").ap()
    bo_d = nc.dram_tensor("bo_r", [P, 4], f32, kind="ExternalInput").ap()
    cm_d = nc.dram_tensor("cmask", [P, 1024], bf, kind="ExternalInput").ap()
    y_d = nc.dram_tensor("yT", [DHALF, T], f32, kind="ExternalOutput").ap()

    x_r = x_d.rearrange("(g p) t -> p g t", p=P)

    with tile.TileContext(nc) as tc, ExitStack() as ctx:
        const = ctx.enter_context(tc.tile_pool(name="const", bufs=1))
        xchunk = ctx.enter_context(tc.tile_pool(name="xchunk", bufs=2))
        expps = ctx.enter_context(tc.tile_pool(name="expps", bufs=4))
        small = ctx.enter_context(tc.tile_pool(name="small", bufs=2))
        afp = ctx.enter_context(tc.tile_pool(name="afp", bufs=4))
        dram = ctx.enter_context(tc.tile_pool(name="dram", bufs=3, space="DRAM"))
        ps_s = ctx.enter_context(tc.tile_pool(name="ps_s", bufs=2, space="PSUM"))
        ps_av = ctx.enter_context(tc.tile_pool(name="ps_av", bufs=1, space="PSUM"))
        ps_m = ctx.enter_context(tc.tile_pool(name="ps_m", bufs=2, space="PSUM"))

        # ---- resident SBUF tensors ----
        wq_sb = const.tile([P, 8, 4, P], bf)     # [k, kg, g, dout]
        wk_sb = const.tile([P, 8, 4, P], bf)
        wv_sb = const.tile([P, 8, DHALF], bf)    # [k, kg, d] (moving operand)
        wo_sb = const.tile([P, 8, 4, P], bf)     # [din, kg, o, dout]
        k_sb = const.tile([P, 4, T], bf)         # k^T per head-pair group
        q_sb = const.tile([P, 4, T], bf)
        v1_sb = const.tile([P, 16, 8, 65], bf)   # [tk, tkblk, head, V|1]
        a_sb = const.tile([P, 4, T], bf)         # attention out (d, tq)
        mk_sb = const.tile([P, 1024], bf)        # diagonal mask
        bq_sb = const.tile([P, 4], f32)
        bo_sb = const.tile([P, 4], f32)

        dma = nc.sync.dma_start
        nc.vector.memset(v1_sb[:, :, :, 64:65], 1.0)

        mm = nc.tensor.matmul

        def _v_group(xc, ts, ti):
            # one 128-token block of V, all 512 head-half dims
            t = 4 * ts + ti
            ps = ps_m.tile([P, DHALF], f32, name="ps", tag="ps")
            for kg in range(8):
                mm(ps, lhsT=xc[:, kg, ti * P:(ti + 1) * P],
                   rhs=wv_sb[:, kg, :], start=(kg == 0), stop=(kg == 7))
            nc.vector.tensor_copy(
                out=v1_sb[:, t, :, 0:64],
                in_=ps.rearrange("p (h c) -> p h c", c=64))

        def _k_group(xc, ts, g):
            ps = ps_m.tile([P, DHALF], f32, name="ps", tag="ps")
            for kg in range(8):
                mm(ps, lhsT=wk_sb[:, kg, g, :], rhs=xc[:, kg, :],
                   start=(kg == 0), stop=(kg == 7))
            nc.vector.tensor_copy(
                out=k_sb[:, g, ts * DHALF:(ts + 1) * DHALF], in_=ps)

        def _q_group(xc, ts, g):
            ps = ps_m.tile([P, DHALF], f32, name="ps", tag="ps")
            for kg in range(8):
                mm(ps, lhsT=wq_sb[:, kg, g, :], rhs=xc[:, kg, :],
                   start=(kg == 0), stop=(kg == 7))
            nc.vector.tensor_scalar_add(
                out=q_sb[:, g, ts * DHALF:(ts + 1) * DHALF], in0=ps,
                scalar1=bq_sb[:, g:g + 1])

        def proj_chunk_thunks(ts):
            # stream 512 tokens of x^T; V blocks 0-1 first (earliest need),
            # then K and Q groups, then V blocks 2-3.
            xc = xchunk.tile([P, 8, DHALF], bf, name="xc", tag="xc")
            dma(out=xc, in_=x_r[:, :, ts * DHALF:(ts + 1) * DHALF])
            th = [partial(_v_group, xc, ts, ti) for ti in (0, 1)]
            th += [partial(_k_group, xc, ts, g) for g in range(4)]
            th += [partial(_q_group, xc, ts, g) for g in range(4)]
            th += [partial(_v_group, xc, ts, ti) for ti in (2, 3)]
            return th

        def attn_slot(g, j):
            # one head-pair group, one 256-query slot, keys 0..2j+1 blocks.
            # One accumulator bank per head; row 64 collects the softmax
            # denominator via the ones column of v1_sb.
            pav = [ps_av.tile([65, 256], f32, tag=f"pav{c}",
                              name=f"pav{c}") for c in (0, 1)]
            last_k = 2 * j + 1
            for kk in range(j + 1):
                kb = (2 * kk, 2 * kk + 1)
                ps = ps_s.tile([P, 1024], f32, name="scps", tag="scps")
                expp = expps.tile([P, 1024], bf, name="expp", tag="expp")
                for c in (0, 1):         # head within pair
                    for i in (0, 1):     # tk block within pair
                        mm(ps[:, c * 512 + i * 256: c * 512 + i * 256 + 256],
                           lhsT=k_sb[64 * c:64 * c + 64, g,
                                     kb[i] * P:(kb[i] + 1) * P],
                           rhs=q_sb[64 * c:64 * c + 64, g,
                                    j * 256:(j + 1) * 256],
                           start=True, stop=True,
                           tile_position=(64 * c, 0))
                nc.scalar.activation(out=expp, in_=ps, func=EXP, scale=0.125)
                if kk == j:
                    nc.vector.tensor_mul(expp, expp, mk_sb)
                for c in (0, 1):
                    for i in (0, 1):
                        mm(pav[c],
                           lhsT=v1_sb[:, kb[i], 2 * g + c, :],
                           rhs=expp[:, c * 512 + i * 256:
                                    c * 512 + i * 256 + 256],
                           start=(kb[i] == 0), stop=(kb[i] == last_k))
            # copy accumulators to SBUF right away so the PSUM banks free up
            av = [small.tile([65, 256], f32, tag=f"av{c}", bufs=2,
                             name=f"av{c}") for c in (0, 1)]
            for c in (0, 1):
                nc.vector.tensor_copy(out=av[c], in_=pav[c])
            # both heads' denominators into one partition-base-0 tile
            # (reciprocal_approx_fast corrupts base!=0 inputs on HW)
            den2 = small.tile([1, 512], f32, tag="den2", bufs=1, name="den2")
            for c in (0, 1):
                nc.vector.tensor_copy(out=den2[:, c * 256:(c + 1) * 256],
                                      in_=av[c][64:65, :])
            rec = small.tile([1, 512], f32, tag="rec", bufs=1, name="rec")
            nc.vector.reciprocal_approx_fast(out=rec, in_=den2)
            sbb = small.tile([64, 512], f32, tag="sbb", name="sbb")
            nc.gpsimd.partition_broadcast(sbb, rec)
            for c in (0, 1):
                nc.vector.tensor_mul(
                    out=a_sb[64 * c:64 * c + 64, g, j * 256:(j + 1) * 256],
                    in0=av[c][0:64, :], in1=sbb[:, c * 256:(c + 1) * 256])

        af_tiles = {}

        def ag_chunk(j):
            # pairwise AllGather of this 256-token chunk of attention out:
            # both ranks end up with [2 ranks x 4 groups] = all 16 heads,
            # in global head order (matches unsliced woT row order).
            inb = dram.tile([P, 1024], bf, name="agin", tag="agin")
            outb = dram.tile([2, P, 1024], bf, name="agout", tag="agout")
            nc.gpsimd.dma_start(
                out=inb.rearrange("p (g q) -> p g q", q=256),
                in_=a_sb[:, :, j * 256:(j + 1) * 256])
            nc.gpsimd.collective_compute(
                "AllGather",
                mybir.AluOpType.bypass,
                replica_groups=[[0, 1], [2, 3], [4, 5], [6, 7]],
                ins=[inb.opt()],
                outs=[outb.opt()],
            )
            af = afp.tile([P, 8, 256], bf, name="af", tag="af")
            dma(out=af.rearrange("p (r g) q -> p r g q", r=2),
                in_=outb.rearrange("r p (g q) -> p r g q", q=256))
            af_tiles[j] = af

        def wo_group(ch, o):
            # 128 output rows x 256 tokens; contraction over all 16 heads
            af = af_tiles[ch]
            ps = ps_m.tile([P, DHALF], f32, name="ps", tag="ps")
            for kg in range(8):
                mm(ps[:, 0:256], lhsT=wo_sb[:, kg, o, :], rhs=af[:, kg, :],
                   start=(kg == 0), stop=(kg == 7))
            ysb = small.tile([P, 256], f32, tag="ysb", name="ysb")
            nc.vector.tensor_scalar_add(out=ysb, in0=ps[:, 0:256],
                                        scalar1=bo_sb[:, o:o + 1])
            dma(out=y_d[o * P:(o + 1) * P, ch * 256:(ch + 1) * 256], in_=ysb)

        def drain(pend, n):
            for _ in range(min(n, len(pend))):
                pend.pop(0)()

        # ---- emission order sets priority ----
        # Warm the exp table set while the first DMAs are in flight, and
        # fire a tiny dummy AllGather so the ~35us first-collective setup
        # (ncfw/NCCL warmup + pair rendezvous) overlaps the projection
        # prologue instead of stalling the first real exchange.
        warm = small.tile([1, 8], f32, tag="warm", bufs=1, name="warm")
        nc.vector.memset(warm, 0.0)
        nc.scalar.activation(out=warm, in_=warm, func=EXP)
        wsb = small.tile([1, 16], bf, tag="wsb", bufs=1, name="wsb")
        nc.vector.memset(wsb, 0.0)
        win = dram.tile([1, 16], bf, name="win", tag="win")
        wout = dram.tile([2, 1, 16], bf, name="wout", tag="wout")
        nc.gpsimd.dma_start(out=win, in_=wsb)
        nc.gpsimd.collective_compute(
            "AllGather", mybir.AluOpType.bypass,
            replica_groups=[[0, 1], [2, 3], [4, 5], [6, 7]],
            ins=[win.opt()], outs=[wout.opt()])

        pend0 = proj_chunk_thunks(0)           # fires the x chunk-0 DMA
        dma(out=wv_sb, in_=wv_d.rearrange("(kg p) d -> p kg d", p=P))
        dma(out=wk_sb, in_=wk_d.rearrange("(kg p) (g c) -> p kg g c", p=P, c=P))
        dma(out=bq_sb, in_=bq_d)
        drain(pend0, 2)                        # V blocks 0-1 of chunk 0
        dma(out=wq_sb, in_=wq_d.rearrange("(kg p) (g c) -> p kg g c", p=P, c=P))
        drain(pend0, 4)                        # K of chunk 0
        dma(out=mk_sb, in_=cm_d)
        dma(out=bo_sb, in_=bo_d)
        drain(pend0, 4)                        # Q of chunk 0
        dma(out=wo_sb, in_=wo_d.rearrange("(kg p) (o c) -> p kg o c", p=P, c=P))

        pend = pend0 + proj_chunk_thunks(1) + proj_chunk_thunks(2) \
            + proj_chunk_thunks(3)
        wo_pend = []
        for j in range(8):
            for g in range(4):
                attn_slot(g, j)
                drain(pend, 2)
                drain(wo_pend, 1)
            ag_chunk(j)
            wo_pend += [partial(wo_group, j, o) for o in range(4)]
        drain(pend, 99)
        drain(wo_pend, 99)

    nc.compile()
    return nc


def _get_nc():
    if "nc" not in _COMPILED:
        _COMPILED["nc"] = _build_nc()
    return _COMPILED["nc"]


def _make_mask():
    tri = np.triu(np.ones((P, P), np.float32))   # keep tk <= tq
    on = np.ones((P, P), np.float32)
    z = np.zeros((P, P), np.float32)
    # expp column layout per head c: [tk 2j vs (q2j | q2j+1) | tk 2j+1 vs ...]
    row = np.concatenate([tri, on, z, tri] * 2, axis=1)
    return row.astype(BF16)


def _make_in_maps(x, wq, bq, wk, bk, wv, bv, wo, bo):
    bfc = lambda a: np.ascontiguousarray(np.asarray(a, np.float32)).astype(BF16)
    bo_eff = (np.asarray(bo, np.float32)
              + np.asarray(wo, np.float32) @ np.asarray(bv, np.float32))
    mask = _make_mask()
    in_maps = []
    for core in range(NCORES):
        b, hh = core // 2, core % 2
        sl = slice(DHALF * hh, DHALF * hh + DHALF)
        m = {
            "xT": bfc(np.asarray(x[b], np.float32).T),
            "wqT": bfc(np.asarray(wq, np.float32)[sl, :].T),
            "wkT": bfc(np.asarray(wk, np.float32)[sl, :].T),
            "wvT": bfc(np.asarray(wv, np.float32)[sl, :].T),
            "woT": bfc(np.asarray(wo, np.float32)[sl, :].T),
            "bq_r": np.ascontiguousarray(
                np.asarray(bq, np.float32)[sl].reshape(4, P).T),
            "bo_r": np.ascontiguousarray(bo_eff[sl].reshape(4, P).T),
            "cmask": mask,
        }
        in_maps.append(m)
    return in_maps


def _run(inputs, trace=False):
    from concourse.bass_utils import run_bass_kernel_spmd
    nc = _get_nc()
    in_maps = _make_in_maps(**inputs)
    res = run_bass_kernel_spmd(nc, in_maps, list(range(NCORES)), trace=trace)
    y = np.empty((B, T, D), np.float32)
    for core in range(NCORES):
        b, hh = core // 2, core % 2
        y[b][:, DHALF * hh:DHALF * hh + DHALF] = res.results[core]["yT"].T
    return y, res


def kernel(**inputs):
    y, _ = _run(inputs, trace=False)
    return y
